# revision 1
# baseline (speedup 1.0000x reference)
"""AttentionBlock (GroupNorm + single-head self-attention + proj + residual)
on 8 trn2 NeuronCores.

Sharding: 8 cores = 4 batch elements x 2 query-halves. Each core computes
GroupNorm + full K/V for its batch element (duplicated across the 2 cores
sharing a batch, ~10% redundant FLOPs) and attention for its half of the
4096 tokens. Token order is rotated per-half on the host so every core runs
the identical NEFF on "its" tokens 0..2047 (SPMD, no collectives).

Device math (per core), all layouts channel-major [c_part, tok_free]:
  x [512, 4096] f32 -> GN stats (per-partition sums + indicator-matmul for
  group sums) -> xn bf16 -> qkvT = W_eff @ xn (GN affine + 1/sqrt(c) folded
  into weights host-side, bf16 matmul, fp32 PSUM) -> S^T tiles [ktok, qtok]
  -> exp on ScalarE (no max subtraction; scores ~ N(0,1), fp32 exp safe) ->
  E^T bf16 -> softmax denominator via DVE accumulation + one ones-matvec
  partition reduce -> attn@V with V in token-major layout -> scale by
  1/denom (DVE reciprocal, gpsimd partition_broadcast) -> proj matmul ->
  + residual(+proj_bias, host-prefolded) -> out [512, 2048] f32.
"""

import sys

if "/opt/trn_rl_repo" not in sys.path:
    sys.path.insert(0, "/opt/trn_rl_repo")

import numpy as np
import ml_dtypes

import concourse.bass as bass
import concourse.bacc as bacc
import concourse.tile as tile
from concourse import mybir
from concourse.bass_utils import run_bass_kernel_spmd

F32 = mybir.dt.float32
BF16 = mybir.dt.bfloat16
AF = mybir.ActivationFunctionType

N, C, H, W = 4, 512, 64, 64
T = H * W            # 4096 tokens
TH = T // 2          # 2048 tokens per core
GROUPS = 32
GSIZE = C // GROUPS  # 16 channels per group
EPS = 1e-5
CT = C // 128        # 4 channel tiles
QB = TH // 512       # 4 query blocks of 512
KT = T // 128        # 32 key-token tiles

_CACHE = {}


def _build(with_qkv_bias: bool):
    nc = bacc.Bacc("TRN2", target_bir_lowering=False, debug=False,
                   enable_asserts=False, num_devices=8)

    x_d = nc.dram_tensor("x", [C, T], F32, kind="ExternalInput")
    wqkv_d = nc.dram_tensor("wqkvT", [C, 3 * C], BF16, kind="ExternalInput")
    wproj_d = nc.dram_tensor("wprojT", [C, C], BF16, kind="ExternalInput")
    resid_d = nc.dram_tensor("resid", [C, TH], F32, kind="ExternalInput")
    ind_d = nc.dram_tensor("ind", [128, 128], F32, kind="ExternalInput")
    if with_qkv_bias:
        qb_d = nc.dram_tensor("qkv_bias", [128, 12], F32, kind="ExternalInput")
    out_d = nc.dram_tensor("out", [C, TH], F32, kind="ExternalOutput")

    with tile.TileContext(nc) as tc:
        with (
            tc.tile_pool(name="const", bufs=1) as cpool,
            tc.tile_pool(name="big", bufs=2) as bigpool,
            tc.tile_pool(name="kv", bufs=1) as kvpool,
            tc.tile_pool(name="small", bufs=4) as spool,
            tc.tile_pool(name="attn", bufs=2) as apool,
            tc.tile_pool(name="io", bufs=3) as iopool,
            tc.tile_pool(name="psA", bufs=4, space="PSUM") as psA,
            tc.tile_pool(name="psB", bufs=3, space="PSUM") as psB,
        ):
            # ---- constants (ind first: it gates the stats matmuls) ----
            ind_sb = cpool.tile([128, 128], F32)
            nc.sync.dma_start(out=ind_sb[:], in_=ind_d[:])
            ones_sb = cpool.tile([128, 1], F32)
            nc.vector.memset(ones_sb[:], 1.0)
            wq_sb = cpool.tile([128, CT, 3 * C], BF16)
            wp_sb = cpool.tile([128, CT, C], BF16)
            if with_qkv_bias:
                qbias_sb = cpool.tile([128, 12], F32)
                nc.sync.dma_start(out=qbias_sb[:], in_=qb_d[:])

            # ---- GroupNorm -> xn (bf16, [128, CT, T]) ----
            # x/sq scratch borrows the kv pool's slots (kt/vt/qt are only
            # written in the qkv phase, after GN is done with the space).
            # Stats run on half-tiles so compute starts as soon as the first
            # DMA chunk lands; normalize is spread across DVE/ACT/GpSimd.
            xn = bigpool.tile([128, CT, T], BF16, tag="big")
            TH2 = T // 2
            if True:
                for ct in range(CT):
                    x_t = kvpool.tile([128, T], F32,
                                      tag=("kt" if ct % 2 == 0 else "vt"))
                    s12h = spool.tile([128, 4], F32, tag="s12")
                    for h in range(2):
                        sl = slice(h * TH2, (h + 1) * TH2)
                        nc.sync.dma_start(out=x_t[:, sl],
                                          in_=x_d[ct * 128:(ct + 1) * 128, sl])
                        sq_t = kvpool.tile([128, TH2], BF16, tag="qt")
                        nc.scalar.activation(sq_t[:], x_t[:, sl], AF.Square,
                                             accum_out=s12h[:, 2 + h:3 + h])
                        nc.vector.reduce_sum(s12h[:, h:h + 1], x_t[:, sl],
                                             axis=mybir.AxisListType.X)
                    s12 = spool.tile([128, 2], F32, tag="s12c")
                    nc.vector.tensor_add(s12[:, 0:1], s12h[:, 0:1], s12h[:, 1:2])
                    nc.vector.tensor_add(s12[:, 1:2], s12h[:, 2:3], s12h[:, 3:4])
                    # group-sum across partitions via indicator matmul
                    ps_pc = psA.tile([128, 2], F32, tag="ps")
                    nc.tensor.matmul(ps_pc[:], ind_sb[:], s12[:],
                                     start=True, stop=True)
                    ms = spool.tile([128, 2], F32, tag="ms")
                    nc.vector.tensor_scalar_mul(ms[:], ps_pc[:],
                                                1.0 / (GSIZE * T))
                    stat = spool.tile([128, 4], F32, tag="stat")
                    mean, var, rstd, nbias = (stat[:, i:i + 1] for i in range(4))
                    nc.vector.tensor_mul(mean, ms[:, 0:1], ms[:, 0:1])
                    nc.vector.tensor_sub(var, ms[:, 1:2], mean)
                    nc.vector.tensor_scalar_add(var, var, EPS)
                    nc.scalar.activation(var, var, AF.Sqrt)
                    nc.vector.reciprocal(rstd, var)
                    nc.vector.tensor_mul(nbias, ms[:, 0:1], rstd)
                    nc.vector.tensor_scalar_mul(nbias, nbias, -1.0)
                    # normalize: split halves across engines to shorten the
                    # serial GN head (DVE / ACT / GpSimd)
                    for h in range(2):
                        sl = slice(h * TH2, (h + 1) * TH2)
                        eng = (ct * 2 + h) % 3
                        if eng == 0:
                            nc.vector.tensor_scalar(
                                xn[:, ct, sl], x_t[:, sl], rstd, nbias,
                                mybir.AluOpType.mult, mybir.AluOpType.add)
                        elif eng == 1:
                            nc.scalar.activation(xn[:, ct, sl], x_t[:, sl],
                                                 AF.Identity,
                                                 bias=nbias, scale=rstd)
                        else:
                            nc.gpsimd.tensor_scalar(
                                xn[:, ct, sl], x_t[:, sl], rstd, nbias,
                                mybir.AluOpType.mult, mybir.AluOpType.add)

            # weights land after x: they are not needed until qkv
            for ct in range(CT):
                nc.sync.dma_start(out=wq_sb[:, ct, :],
                                  in_=wqkv_d[ct * 128:(ct + 1) * 128, :])
            for ct in range(CT):
                nc.sync.dma_start(out=wp_sb[:, ct, :],
                                  in_=wproj_d[ct * 128:(ct + 1) * 128, :])

            # ---- qkv projections ----
            # kT [c_head, tok] and qT [c_head, tok(half)], channel-major
            kt_sb = kvpool.tile([128, CT, T], BF16, tag="kt")
            qt_sb = kvpool.tile([128, CT, TH], BF16, tag="qt")
            vt_sb = kvpool.tile([128, KT, C], BF16, tag="vt")
            for dk in range(CT):     # kT: qkv rows 512..1023
                for ts in range(T // 512):
                    ps = psA.tile([128, 512], F32, tag="ps")
                    for ct in range(CT):
                        nc.tensor.matmul(
                            ps[:],
                            wq_sb[:, ct, C + dk * 128: C + (dk + 1) * 128],
                            xn[:, ct, ts * 512:(ts + 1) * 512],
                            start=(ct == 0), stop=(ct == CT - 1))
                    if with_qkv_bias:
                        nc.scalar.activation(kt_sb[:, dk, ts * 512:(ts + 1) * 512],
                                             ps[:], AF.Identity,
                                             bias=qbias_sb[:, 4 + dk:5 + dk])
                    else:
                        nc.scalar.copy(kt_sb[:, dk, ts * 512:(ts + 1) * 512], ps[:])
            for dq in range(CT):     # qT: qkv rows 0..511, first TH tokens
                for ts in range(TH // 512):
                    ps = psA.tile([128, 512], F32, tag="ps")
                    for ct in range(CT):
                        nc.tensor.matmul(
                            ps[:],
                            wq_sb[:, ct, dq * 128:(dq + 1) * 128],
                            xn[:, ct, ts * 512:(ts + 1) * 512],
                            start=(ct == 0), stop=(ct == CT - 1))
                    if with_qkv_bias:
                        nc.scalar.activation(qt_sb[:, dq, ts * 512:(ts + 1) * 512],
                                             ps[:], AF.Identity,
                                             bias=qbias_sb[:, dq:dq + 1])
                    else:
                        nc.scalar.copy(qt_sb[:, dq, ts * 512:(ts + 1) * 512], ps[:])
            for tv in range(KT):     # V token-major [tok, c], qkv rows 1024..1535
                ps = psA.tile([128, 512], F32, tag="ps")
                for ct in range(CT):
                    nc.tensor.matmul(
                        ps[:],
                        xn[:, ct, tv * 128:(tv + 1) * 128],
                        wq_sb[:, ct, 2 * C:3 * C],
                        start=(ct == 0), stop=(ct == CT - 1))
                nc.vector.tensor_copy(vt_sb[:, tv, :], ps[:])

            # ---- attention, per query block of 512 ----
            for qb in range(QB):
                et = bigpool.tile([128, KT, 512], BF16, tag="big")
                acc = apool.tile([128, 512], F32, tag="acc")
                for kt in range(KT):
                    ps_st = psA.tile([128, 512], F32, tag="ps")
                    for cd in range(CT):
                        nc.tensor.matmul(
                            ps_st[:],
                            kt_sb[:, cd, kt * 128:(kt + 1) * 128],
                            qt_sb[:, cd, qb * 512:(qb + 1) * 512],
                            start=(cd == 0), stop=(cd == CT - 1))
                    nc.scalar.activation(et[:, kt, :], ps_st[:], AF.Exp)
                    if kt == 0:
                        nc.vector.tensor_copy(acc[:], et[:, 0, :])
                    else:
                        nc.vector.tensor_add(acc[:], acc[:], et[:, kt, :])
                # partition-reduce the per-partition sums, then 1/x broadcast
                ps_den = psA.tile([1, 512], F32, tag="ps")
                nc.tensor.matmul(ps_den[:], ones_sb[:], acc[:],
                                 start=True, stop=True)
                den_sb = spool.tile([1, 512], F32, tag="den")
                nc.vector.tensor_copy(den_sb[:], ps_den[:])
                rbd = apool.tile([128, 512], F32, tag="rbd")
                nc.gpsimd.partition_broadcast(rbd[:], den_sb[:])
                rb = apool.tile([128, 512], F32, tag="rb")
                nc.vector.reciprocal(rb[:], rbd[:])
                at_sb = apool.tile([128, CT, 512], BF16, tag="at")
                for cv in range(CT):
                    ps_av = psB.tile([128, 512], F32, tag="av")
                    for kt in range(KT):
                        nc.tensor.matmul(
                            ps_av[:],
                            vt_sb[:, kt, cv * 128:(cv + 1) * 128],
                            et[:, kt, :],
                            start=(kt == 0), stop=(kt == KT - 1))
                    nc.vector.tensor_mul(at_sb[:, cv, :], ps_av[:], rb[:])
                # proj + residual
                for co in range(CT):
                    ps_pr = psA.tile([128, 512], F32, tag="ps")
                    for ci in range(CT):
                        nc.tensor.matmul(
                            ps_pr[:],
                            wp_sb[:, ci, co * 128:(co + 1) * 128],
                            at_sb[:, ci, :],
                            start=(ci == 0), stop=(ci == CT - 1))
                    r_t = iopool.tile([128, 512], F32, tag="r")
                    nc.sync.dma_start(
                        out=r_t[:],
                        in_=resid_d[co * 128:(co + 1) * 128,
                                    qb * 512:(qb + 1) * 512])
                    o_t = iopool.tile([128, 512], F32, tag="o")
                    nc.vector.tensor_add(o_t[:], ps_pr[:], r_t[:])
                    nc.sync.dma_start(
                        out=out_d[co * 128:(co + 1) * 128,
                                  qb * 512:(qb + 1) * 512],
                        in_=o_t[:])

    nc.compile()
    return nc


def _prep_inputs(x, gn_weight, gn_bias, qkv_weight, proj_weight, proj_bias):
    """Host-side shard prep. Returns (in_maps, with_qkv_bias)."""
    bf16 = ml_dtypes.bfloat16
    x, gn_weight, gn_bias, qkv_weight, proj_weight, proj_bias = (
        np.asarray(a) for a in
        (x, gn_weight, gn_bias, qkv_weight, proj_weight, proj_bias))
    xr = np.ascontiguousarray(x.reshape(N, C, T).astype(np.float32))
    scale = 1.0 / np.sqrt(C)
    w_eff = qkv_weight.astype(np.float64) * gn_weight.astype(np.float64)[None, :]
    w_eff[:C] *= scale  # fold attention score scale into q
    qkv_bias = (qkv_weight.astype(np.float64) @ gn_bias.astype(np.float64))
    qkv_bias[:C] *= scale
    with_qkv_bias = bool(np.any(qkv_bias != 0.0))
    wqkvT = np.ascontiguousarray(w_eff.T.astype(bf16))          # [C, 3C]
    wprojT = np.ascontiguousarray(proj_weight.T.astype(bf16))   # [C, C]
    ind = (np.arange(128)[:, None] // GSIZE ==
           np.arange(128)[None, :] // GSIZE).astype(np.float32)
    in_maps = []
    for core in range(8):
        b, half = divmod(core, 2)
        xb = xr[b]
        if half:
            xb = np.ascontiguousarray(np.roll(xb, -TH, axis=1))
        resid = (xr[b][:, half * TH:(half + 1) * TH]
                 + proj_bias.astype(np.float32)[:, None])
        m = {"x": xb, "wqkvT": wqkvT, "wprojT": wprojT,
             "resid": np.ascontiguousarray(resid.astype(np.float32)),
             "ind": ind}
        if with_qkv_bias:
            m["qkv_bias"] = np.ascontiguousarray(
                qkv_bias.astype(np.float32).reshape(12, 128).T)
        in_maps.append(m)
    return in_maps, with_qkv_bias


def kernel(x, gn_weight, gn_bias, qkv_weight, proj_weight, proj_bias,
           _trace=False):
    in_maps, with_qkv_bias = _prep_inputs(
        x, gn_weight, gn_bias, qkv_weight, proj_weight, proj_bias)
    if with_qkv_bias not in _CACHE:
        _CACHE[with_qkv_bias] = _build(with_qkv_bias)
    nc = _CACHE[with_qkv_bias]
    res = run_bass_kernel_spmd(nc, in_maps, core_ids=list(range(8)),
                               trace=_trace)
    kernel.last_results = res
    out = np.empty((N, C, T), np.float32)
    for core in range(8):
        b, half = divmod(core, 2)
        out[b][:, half * TH:(half + 1) * TH] = res.results[core]["out"]
    return out.reshape(N, C, H, W)

